# revision 1
# baseline (speedup 1.0000x reference)
"""GAT-with-LSTM-gates kernel for Trainium2, SPMD over 8 NeuronCores.

Problem: B=16 graphs, N=1024 nodes, D=128 features.
    h   = x @ Ww.T + Wb
    e   = (h @ A) @ h.T;  e_sym = e + e.T  (== h @ (A + A.T) @ h.T)
    s   = where(adj > 0, e_sym, 0)
    att = softmax(s, axis=1) * adj
    h'  = relu(att @ h)
    ic/fc/oc = sigmoid(h' @ w*_u + x @ w*_x)        (scalar per node)
    out = oc * tanh(ic * h' + fc * x)

Sharding: data-parallel over B; 2 graphs per core; params replicated.

Device-side formulation (per graph), all in "transposed" layouts so that
the softmax axis is the free dimension:
    hT[d, n]    = Ww @ x.T + Wb               (matmul, lhsT = Ww.T)
    hAsT[l, n]  = (A + A.T) @ hT              (matmul, lhsT = As)
    e[c, a]     = e_sym[c, a]  (symmetric)    (matmul, lhsT = hAsT c-slice)
    p[c, a]     = exp(e[c, a])                (no max-shift: |e| <~ 20)
    q[c, a]     = p * adjT                    (adjT = adj.T, bf16, exact 0/1)
    Z[c]        = sum_a q[c, a] + (N - deg[c])   (deg = rowsum(adjT);
                   masked entries contribute exp(0)=1 to the reference
                   softmax denominator)
    h'T[d, a]   = sum_c (h[c, d]/Z[c]) * q[c, a]  (1/Z folded into the
                   small h matrix, not the [N,N] attention matrix)
    h'T         = relu(h'T)
    GT[3, n]    = U.T @ h'T + Xw.T @ xT;  gates = sigmoid(GT)
                   (sigmoid via 0.5*tanh(0.5 z)+0.5 to stay in the exp/tanh
                   ACT table set)
    out[a, d]   = oc * tanh(ic * h'_nat + fc * x_nat)
"""

import numpy as np

import concourse.bacc as bacc
import concourse.bass as bass
import concourse.mybir as mybir
import concourse.tile as tile
from concourse.bass_utils import run_bass_kernel_spmd

F32 = mybir.dt.float32
BF16 = mybir.dt.bfloat16
AF = mybir.ActivationFunctionType
OP = mybir.AluOpType

B, N, D = 16, 1024, 128
NCORES = 8
GPC = B // NCORES  # graphs per core
NC_TILES = N // 128  # 8 column/row tiles of the [N, N] score matrix

# How many of the 8 mask-multiply (q = p * adjT) tiles run on GPSIMD
# instead of DVE (load balance between the two engines).
Q_TILES_ON_GPSIMD = 3


def _build_program(reps=1):
    """reps>1 wraps the whole per-call body in a hardware loop — used only
    for benchmarking (amortizes the host->device dispatch overhead)."""
    nc = bacc.Bacc(None, enable_partition_id=False)

    xT = nc.dram_tensor("xT", [GPC, D, N], F32, kind="ExternalInput")
    xn = nc.dram_tensor("xn", [GPC, N, D], F32, kind="ExternalInput")
    adjT = nc.dram_tensor("adjT", [GPC, N, N], BF16, kind="ExternalInput")
    # all replicated params in one tensor -> one DMA -> one sync wait
    # columns: [WwT(128) | As(128) | I128(128) | Wb(1) | U(3) | Xw(3)]
    consts_d = nc.dram_tensor("consts", [D, 391], F32, kind="ExternalInput")
    out = nc.dram_tensor("out", [GPC, N, D], F32, kind="ExternalOutput")

    with tile.TileContext(nc) as tc:
        with (
            tc.tile_pool(name="const", bufs=1) as constp,
            tc.tile_pool(name="big", bufs=2) as big,
            tc.tile_pool(name="adjp", bufs=3) as adjp,
            tc.tile_pool(name="qp", bufs=3) as qp,
            tc.tile_pool(name="small", bufs=2) as small,
            tc.tile_pool(name="ps_big", bufs=2, space="PSUM") as ps_big,
            tc.tile_pool(name="ps_hp", bufs=1, space="PSUM") as ps_hp,
            tc.tile_pool(name="ps_small", bufs=2, space="PSUM") as ps_small,
        ):
            # ---- constants (loaded once, single DMA) ----
            consts = constp.tile([D, 391], F32, name="consts_sb")
            nc.sync.dma_start(out=consts[:], in_=consts_d[:])
            WwT = consts[:, 0:128]
            As = consts[:, 128:256]
            I128 = consts[:, 256:384]
            Wb = consts[:, 384:385]
            U = consts[:, 385:388]
            Xw = consts[:, 388:391]

            import contextlib
            loop_ctx = (tc.For_i(0, reps, 1) if reps > 1
                        else contextlib.nullcontext())
            with loop_ctx:
              for g in range(GPC):
                # ---- load x in both layouts ----
                xT_sb = big.tile([D, N], F32, name="xT_sb", tag="xT")
                nc.sync.dma_start(out=xT_sb[:], in_=xT[g])

                # ---- hT = Ww @ x.T + Wb ----
                hT_ps = ps_big.tile([D, 2, 512], F32, name="hT_ps", tag="bigps")
                for k in range(2):
                    nc.tensor.matmul(
                        hT_ps[:, k, :], WwT[:], xT_sb[:, k * 512 : (k + 1) * 512],
                        start=True, stop=True,
                    )
                hT = big.tile([D, N], F32, name="hT", tag="hT")
                for k in range(2):
                    nc.scalar.activation(
                        hT[:, k * 512 : (k + 1) * 512], hT_ps[:, k, :],
                        AF.Identity, bias=Wb[:],
                    )

                # ---- hAsT = (A + A.T) @ hT ----
                hAsT_ps = ps_big.tile([D, 2, 512], F32, name="hAsT_ps", tag="bigps")
                for k in range(2):
                    nc.tensor.matmul(
                        hAsT_ps[:, k, :], As[:], hT[:, k * 512 : (k + 1) * 512],
                        start=True, stop=True,
                    )
                hAsT = big.tile([D, N], F32, name="hAsT", tag="hAsT")
                for k in range(2):
                    nc.vector.tensor_copy(
                        hAsT[:, k * 512 : (k + 1) * 512], hAsT_ps[:, k, :]
                    )

                # ---- h in natural layout: h_nd[:, ci, :] = h[128ci:128ci+128, :]
                h_nd = big.tile([128, NC_TILES, D], F32, name="h_nd", tag="h_nd")
                h_s = big.tile([128, NC_TILES, D], F32, name="h_s", tag="h_s")
                for ci in range(NC_TILES):
                    tr_ps = ps_small.tile([128, D], F32, name="tr_ps", tag="smallps")
                    nc.tensor.transpose(
                        tr_ps[:], hT[:, ci * 128 : (ci + 1) * 128], I128[:]
                    )
                    nc.vector.tensor_copy(h_nd[:, ci, :], tr_ps[:])

                # ---- attention: per 128-row strip of the score matrix ----
                hp_ps = ps_hp.tile([D, 2, 512], F32, name="hp_ps", tag="hpps")
                for ci in range(NC_TILES):
                    adj_sb = adjp.tile([128, N], BF16, name="adj_sb", tag="adj")
                    nc.sync.dma_start(
                        out=adj_sb[:], in_=adjT[g, ci * 128 : (ci + 1) * 128, :]
                    )
                    # deg -> nz = N - deg  (seed of the Z reduction)
                    nz = small.tile([128, 1], F32, name="nz", tag="nz")
                    nc.vector.tensor_reduce(
                        nz[:], adj_sb[:], mybir.AxisListType.X, OP.add
                    )
                    nc.vector.tensor_scalar(
                        nz[:], nz[:], -1.0, float(N), OP.mult, OP.add
                    )

                    e_ps = ps_big.tile([128, 2, 512], F32, name="e_ps", tag="bigps")
                    for k in range(2):
                        nc.tensor.matmul(
                            e_ps[:, k, :],
                            hAsT[:, ci * 128 : (ci + 1) * 128],
                            hT[:, k * 512 : (k + 1) * 512],
                            start=True, stop=True,
                        )
                    p_sb = qp.tile([128, N], F32, name="p_sb", tag="p")
                    nc.scalar.activation(
                        p_sb[:], e_ps.rearrange("p a b -> p (a b)"), AF.Exp
                    )

                    q_sb = qp.tile([128, N], F32, name="q_sb", tag="q")
                    Z = small.tile([128, 1], F32, name="Z", tag="Z")
                    if ci < Q_TILES_ON_GPSIMD:
                        nc.gpsimd.tensor_tensor(q_sb[:], p_sb[:], adj_sb[:], OP.mult)
                        nc.vector.tensor_reduce(
                            Z[:], q_sb[:], mybir.AxisListType.X, OP.add
                        )
                    else:
                        # q = (p * 1) * adjT with fused row-sum
                        nc.vector.scalar_tensor_tensor(
                            out=q_sb[:], in0=p_sb[:], scalar=1.0, in1=adj_sb[:],
                            op0=OP.mult, op1=OP.mult, accum_out=Z[:],
                        )
                    nc.vector.tensor_scalar(Z[:], Z[:], nz[:], None, OP.add)
                    R = small.tile([128, 1], F32, name="R", tag="R")
                    nc.vector.reciprocal(R[:], Z[:])
                    nc.vector.tensor_scalar(
                        h_s[:, ci, :], h_nd[:, ci, :], R[:], None, OP.mult
                    )
                    # accumulate h'T += h_s[ci].T @ q[ci]
                    for k in range(2):
                        nc.tensor.matmul(
                            hp_ps[:, k, :],
                            h_s[:, ci, :],
                            q_sb[:, k * 512 : (k + 1) * 512],
                            start=(ci == 0), stop=(ci == NC_TILES - 1),
                        )

                # ---- h' = relu ----
                hp = big.tile([D, N], F32, name="hp", tag="hp")
                for k in range(2):
                    nc.scalar.activation(
                        hp[:, k * 512 : (k + 1) * 512], hp_ps[:, k, :], AF.Relu
                    )

                # ---- gates: GT = U.T @ h'T + Xw.T @ xT; sigmoid via tanh ----
                gt = small.tile([32, N], F32, name="gt", tag="gt")
                for k in range(2):
                    gt_ps = ps_small.tile([128, 512], F32, name="gt_ps", tag="smallps")
                    nc.tensor.matmul(
                        gt_ps[0:3, :], U[:], hp[:, k * 512 : (k + 1) * 512],
                        start=True, stop=False,
                    )
                    nc.tensor.matmul(
                        gt_ps[0:3, :], Xw[:], xT_sb[:, k * 512 : (k + 1) * 512],
                        start=False, stop=True,
                    )
                    nc.scalar.activation(
                        gt[0:3, k * 512 : (k + 1) * 512], gt_ps[0:3, :],
                        AF.Tanh, scale=0.5,
                    )
                nc.vector.tensor_scalar(
                    gt[0:3, :], gt[0:3, :], 0.5, 0.5, OP.mult, OP.add
                )

                # ---- final elementwise stage, in natural [node, feat] layout --
                w_all = big.tile([128, N], F32, name="w_all", tag="w_all")
                t_all = big.tile([128, N], F32, name="t_all", tag="t_all")
                out_sb = big.tile([128, N], F32, name="out_sb", tag="out_sb")
                gn = small.tile([128, 3 * NC_TILES], F32, name="gn", tag="gn")
                for ai in range(NC_TILES):
                    # gates for this node block: [128, 3] (ic, fc, oc columns)
                    g_ps = ps_small.tile([128, 512], F32, name="g_ps", tag="smallps")
                    nc.tensor.transpose(
                        g_ps[:, 0:3], gt[0:3, ai * 128 : (ai + 1) * 128],
                        I128[0:3, 0:3],
                    )
                    nc.vector.tensor_copy(
                        gn[:, ai * 3 : ai * 3 + 3], g_ps[:, 0:3]
                    )
                    xn_sb = small.tile([128, D], F32, name="xn_sb", tag="xn")
                    nc.sync.dma_start(
                        out=xn_sb[:], in_=xn[g, ai * 128 : (ai + 1) * 128, :]
                    )
                    hp_nat = ps_small.tile([128, 512], F32, name="hp_nat", tag="smallps")
                    nc.tensor.transpose(
                        hp_nat[:, 0:D], hp[:, ai * 128 : (ai + 1) * 128], I128[:]
                    )
                    v = small.tile([128, D], F32, name="v", tag="v")
                    nc.gpsimd.tensor_scalar(
                        v[:], xn_sb[:], gn[:, ai * 3 + 1 : ai * 3 + 2], None, OP.mult
                    )
                    nc.vector.scalar_tensor_tensor(
                        out=w_all[:, ai * 128 : (ai + 1) * 128],
                        in0=hp_nat[:, 0:D],
                        scalar=gn[:, ai * 3 : ai * 3 + 1],
                        in1=v[:],
                        op0=OP.mult, op1=OP.add,
                    )
                nc.scalar.activation(t_all[:], w_all[:], AF.Tanh)
                for ai in range(NC_TILES):
                    nc.gpsimd.tensor_scalar(
                        out_sb[:, ai * 128 : (ai + 1) * 128],
                        t_all[:, ai * 128 : (ai + 1) * 128],
                        gn[:, ai * 3 + 2 : ai * 3 + 3], None, OP.mult,
                    )
                    nc.sync.dma_start(
                        out=out[g, ai * 128 : (ai + 1) * 128, :],
                        in_=out_sb[:, ai * 128 : (ai + 1) * 128],
                    )
    nc.finalize()
    return nc


_CACHE = {}


def _get_program():
    if "nc" not in _CACHE:
        _CACHE["nc"] = _build_program()
    return _CACHE["nc"]


def _make_consts(inputs):
    A_ = np.asarray(inputs["A"], np.float32)
    return np.ascontiguousarray(np.concatenate([
        np.asarray(inputs["Ww"], np.float32).T,
        A_ + A_.T,
        np.eye(D, dtype=np.float32),
        np.asarray(inputs["Wb"], np.float32).reshape(D, 1),
        np.stack([inputs["wi_u"], inputs["wf_u"], inputs["wo_u"]],
                 axis=1).astype(np.float32),
        np.stack([inputs["wi_x"], inputs["wf_x"], inputs["wo_x"]],
                 axis=1).astype(np.float32),
    ], axis=1))


def kernel(x, adj, Ww, Wb, A, wi_u, wi_x, wf_u, wf_x, wo_u, wo_x):
    x = np.ascontiguousarray(np.asarray(x, dtype=np.float32))
    adj = np.asarray(adj, dtype=np.float32)

    bf16 = mybir.dt.np(BF16)
    # layout prep (host): transposes / stacking / dtype cast only
    xT_all = np.ascontiguousarray(x.transpose(0, 2, 1))           # [B, D, N]
    adjT_all = np.ascontiguousarray(
        adj.transpose(0, 2, 1)).astype(bf16)                      # [B, N, N]
    A_ = np.asarray(A, np.float32)
    consts = np.concatenate([
        np.asarray(Ww, np.float32).T,
        A_ + A_.T,
        np.eye(D, dtype=np.float32),
        np.asarray(Wb, np.float32).reshape(D, 1),
        np.stack([wi_u, wf_u, wo_u], axis=1).astype(np.float32),
        np.stack([wi_x, wf_x, wo_x], axis=1).astype(np.float32),
    ], axis=1)
    consts = np.ascontiguousarray(consts)

    nc = _get_program()
    in_maps = []
    for c in range(NCORES):
        s = slice(c * GPC, (c + 1) * GPC)
        in_maps.append({
            "xT": xT_all[s],
            "xn": x[s],
            "adjT": adjT_all[s],
            "consts": consts,
        })
    res = run_bass_kernel_spmd(nc, in_maps, list(range(NCORES)))
    out = np.empty((B, N, D), dtype=np.float32)
    for c in range(NCORES):
        out[c * GPC : (c + 1) * GPC] = res.results[c]["out"]
    return out



# revision 29
# speedup vs baseline: 10.6668x; 10.6668x over previous
"""GAT-with-LSTM-gates kernel for Trainium2, SPMD over 8 NeuronCores.

Problem: B=16 graphs, N=1024 nodes, D=128 features.
    h   = x @ Ww.T + Wb
    e   = (h @ A) @ h.T;  e_sym = e + e.T  (== h @ (A + A.T) @ h.T)
    s   = where(adj > 0, e_sym, 0)
    att = softmax(s, axis=1) * adj
    h'  = relu(att @ h)
    ic/fc/oc = sigmoid(h' @ w*_u + x @ w*_x)        (scalar per node)
    out = oc * tanh(ic * h' + fc * x)

Sharding: data-parallel over B; 2 graphs per core; params replicated.

Device-side formulation (per graph), transposed layouts so the softmax
axis is the free dimension.  All matmuls run at 1 PE cycle/row: fp16 for
the value path (score precision), bf16 where exp range is needed.
    hT[d, n]    = C1 @ xT + Wb           (C1 = Ww)
    hAsT[l, n]  = C2 @ xT + b2           (C2 = (A+A.T)@Ww, b2 = (A+A.T)@Wb,
                                          host-folded: no hT->hAsT dependency)
    e[c, a]     = hAsT_strip.T @ hT + 50*I @ (adjT-1)    (PSUM f32; the
                   second matmul injects an additive -50 mask so no
                   separate mask multiply is needed)
    q[c, a]     = exp(e)                 (ACT, bf16 out, range ~e^25; the
                   ACT accumulator emits Qs[c] = sum_a q[c, a] for free)
    Z[c]        = Qs[c] + (N - deg[c])   (nz host-precomputed)
    hs[c, d]    = h_nat[c, d] / Z[c]     (fused with the PSUM->SBUF move of
                   the hT-strip transposes; bf16: values can be ~1e-14)
    h'T[d, a]   = sum_c hs[c, d] q[c, a]    (PSUM accumulate over strips)
    gates (natural orientation, per node-strip, 3-wide matmuls):
        gz[a, j] = hp_strip.T @ U + xT_strip.T @ Xw
        gn       = 0.5 * tanh(0.5 * gz) + 0.5
    out[a, d]   = oc * tanh(ic * h'_nat + fc * x_nat)
"""

import numpy as np

import concourse.bacc as bacc
import concourse.bass as bass
import concourse.mybir as mybir
import concourse.tile as tile
from concourse.bass_utils import run_bass_kernel_spmd

F32 = mybir.dt.float32
F16 = mybir.dt.float16
BF16 = mybir.dt.bfloat16
AF = mybir.ActivationFunctionType
OP = mybir.AluOpType

B, N, D = 16, 1024, 128
NCORES = 8
GPC = B // NCORES  # graphs per core
NT = N // 128  # 8 column/row tiles of the [N, N] score matrix


def _build_program(reps=1, unroll=1):
    """reps>1 wraps the whole per-call body in a hardware loop — used only
    for benchmarking (amortizes the host->device dispatch overhead).
    unroll>1 repeats the body inline (used by the timeline simulator to
    estimate the steady-state period, since it cannot follow For_i)."""
    nc = bacc.Bacc(None, enable_partition_id=False)

    xT = nc.dram_tensor("xT", [GPC, D, N], F16, kind="ExternalInput")
    xn = nc.dram_tensor("xn", [GPC, 128, NT, D], F16, kind="ExternalInput")
    # adjm = adj.T - 1 (values -1/0): the e-score mask is injected on the PE
    # as e += 50*I @ adjm, i.e. -50 on masked entries
    adjm = nc.dram_tensor("adjm", [GPC, 128, NT, N], F16, kind="ExternalInput")
    nzd = nc.dram_tensor("nz", [GPC, 128, NT], F32, kind="ExternalInput")
    # replicated params: fp16 [C1T | C2T | I128 | 50*I | U | Xw], f32 [Wb | b2]
    ch = nc.dram_tensor("consts_h", [D, 518], F16, kind="ExternalInput")
    cf = nc.dram_tensor("consts_f", [D, 3], F32, kind="ExternalInput")
    out = nc.dram_tensor("out", [GPC, 128, NT, D], F32, kind="ExternalOutput")

    with tile.TileContext(nc) as tc:
        with (
            tc.tile_pool(name="const", bufs=1) as constp,
            tc.tile_pool(name="big", bufs=2) as big,
            tc.tile_pool(name="adjp", bufs=2) as adjp,
            tc.tile_pool(name="small", bufs=2) as small,
            tc.tile_pool(name="ps_a", bufs=2, space="PSUM") as ps_a,
            tc.tile_pool(name="ps_b", bufs=2, space="PSUM") as ps_b,
        ):
            consts = constp.tile([D, 518], F16, name="ch_sb")
            nc.sync.dma_start(out=consts[:], in_=ch[:])
            constf = constp.tile([D, 3], F32, name="cf_sb")
            nc.sync.dma_start(out=constf[:], in_=cf[:])
            C1T = consts[:, 0:128]
            C2T = consts[:, 128:256]
            I128 = consts[:, 256:384]
            I50 = consts[:, 384:512]
            U = consts[:, 512:515]
            Xw = consts[:, 515:518]
            Wb = constf[:, 0:1]
            b2 = constf[:, 1:2]

            def stage_a(g):
                """DMAs, hT/hAsT, h-natural transposes."""
                t = {}
                t["xT"] = big.tile([D, N], F16, name="xT_sb", tag="xT")
                nc.sync.dma_start(out=t["xT"][:], in_=xT[g])
                t["adj"] = adjp.tile([128, NT, N], F16, name="adj_sb", tag="adj")
                nc.sync.dma_start(out=t["adj"][:, 0:2, :], in_=adjm[g, :, 0:2, :])
                t["nz"] = small.tile([128, NT], F32, name="nz", tag="nz")
                nc.sync.dma_start(out=t["nz"][:], in_=nzd[g])
                nc.sync.dma_start(out=t["adj"][:, 2:5, :], in_=adjm[g, :, 2:5, :])
                nc.sync.dma_start(out=t["adj"][:, 5:8, :], in_=adjm[g, :, 5:8, :])
                t["xn"] = big.tile([128, NT, D], F16, name="xn_sb", tag="xn")
                nc.sync.dma_start(out=t["xn"][:], in_=xn[g])

                hT_ps = ps_a.tile([D, 2, 512], F32, name="hT_ps", tag="psa")
                for k in range(2):
                    nc.tensor.matmul(
                        hT_ps[:, k, :], C1T[:],
                        t["xT"][:, k * 512 : (k + 1) * 512],
                        start=True, stop=True,
                    )
                t["hT"] = big.tile([D, N], F16, name="hT", tag="hT")
                nc.vector.tensor_scalar(
                    t["hT"].rearrange("p (a b) -> p a b", a=2), hT_ps[:],
                    Wb[:], None, OP.add,
                )
                hAsT_ps = ps_a.tile([D, 2, 512], F32, name="hAsT_ps", tag="psa")
                for k in range(2):
                    nc.tensor.matmul(
                        hAsT_ps[:, k, :], C2T[:],
                        t["xT"][:, k * 512 : (k + 1) * 512],
                        start=True, stop=True,
                    )
                t["hAsT"] = big.tile([D, N], F16, name="hAsT", tag="hAsT")
                nc.vector.tensor_scalar(
                    t["hAsT"].rearrange("p (a b) -> p a b", a=2), hAsT_ps[:],
                    b2[:], None, OP.add,
                )
                # h natural (lhsT of the h' contraction): the PSUM->SBUF
                # move is fused with the 1/Z scale in stage_b
                t["tr"] = ps_b.tile([128, NT, D], F16, name="tr_ps", tag="trh")
                for ci in range(NT):
                    nc.tensor.transpose(
                        t["tr"][:, ci, :],
                        t["hT"][:, ci * 128 : (ci + 1) * 128], I128[:],
                    )
                return t

            def emit_hp(t, ci, k):
                nc.tensor.matmul(
                    t["hp_ps"][:],
                    t["hs"][:, ci, :],
                    t["q"][:, ci, k * 512 : (k + 1) * 512],
                    start=(ci == 0), stop=(ci == NT - 1),
                )

            def stage_b(t, mid_hook=None):
                """Attention strips; h' half-0 matmuls one strip behind.
                mid_hook() is emitted after strip 2 so the next graph's
                setup work fills this graph's per-strip engine gaps."""
                t["Qs"] = small.tile([128, NT], F32, name="Qs", tag="Qs")
                t["Z"] = small.tile([128, NT], F32, name="Z", tag="Z")
                t["R"] = small.tile([128, NT], F32, name="R", tag="R")
                t["hs"] = big.tile([128, NT, D], BF16, name="hs", tag="hs")
                t["q"] = big.tile([128, NT, N], BF16, name="q_sb", tag="q")
                t["hp_ps"] = ps_b.tile([D, 512], F32, name="hp_ps", tag="pshp")
                t["hp"] = big.tile([D, N], F16, name="hp", tag="hp")
                for ci in range(NT):
                    e_ps = ps_a.tile([128, 2, 512], F32, name="e_ps", tag="psa")
                    for k in range(2):
                        nc.tensor.matmul(
                            e_ps[:, k, :],
                            t["hAsT"][:, ci * 128 : (ci + 1) * 128],
                            t["hT"][:, k * 512 : (k + 1) * 512],
                            start=True, stop=False,
                        )
                        # additive mask: e += 50*I @ (adjT-1)
                        nc.tensor.matmul(
                            e_ps[:, k, :],
                            I50[:],
                            t["adj"][:, ci, k * 512 : (k + 1) * 512],
                            start=False, stop=True, skip_group_check=True,
                        )
                    if ci > 0:
                        emit_hp(t, ci - 1, 0)
                    # q = exp(e - 50*(1-adj)); row-sums via the ACT accumulator
                    nc.scalar.activation(
                        t["q"][:, ci, :], e_ps.rearrange("p a b -> p (a b)"),
                        AF.Exp, accum_out=t["Qs"][:, ci : ci + 1],
                    )
                    nc.vector.tensor_scalar(
                        t["Z"][:, ci : ci + 1], t["Qs"][:, ci : ci + 1],
                        t["nz"][:, ci : ci + 1], None, OP.add,
                    )
                    nc.vector.reciprocal(
                        t["R"][:, ci : ci + 1], t["Z"][:, ci : ci + 1])
                    # hs = h_nat * (1/Z): fused PSUM->SBUF move + scale
                    nc.vector.tensor_scalar(
                        t["hs"][:, ci, :], t["tr"][:, ci, :],
                        t["R"][:, ci : ci + 1], None, OP.mult,
                    )
                    if ci == 2 and mid_hook is not None:
                        t["_next"] = mid_hook()
                emit_hp(t, NT - 1, 0)

            def relu_half(t, k):
                nc.vector.tensor_scalar(
                    t["hp"][:, k * 512 : (k + 1) * 512], t["hp_ps"][:],
                    0.0, None, OP.max,
                )

            def stage_c(t, g, relu0_done):
                """h' half 1, gates, final elementwise stage, output."""
                if not relu0_done:
                    relu_half(t, 0)
                gps = ps_b.tile([128, 32], F32, name="gps", tag="pshp")
                gn = small.tile([128, 3 * NT], F32, name="gn", tag="gn")
                hpn = ps_b.tile([128, NT, D], F16, name="hpn", tag="trh")
                v_all = big.tile([128, NT, D], F16, name="v_all", tag="v")
                w_all = big.tile([128, NT, D], F16, name="w_all", tag="w")
                t_all = big.tile([128, NT, D], F16, name="t_all", tag="t")
                out_sb = big.tile([128, NT, D], F32, name="out_sb", tag="o")
                for k in range(2):
                    if k == 1:
                        relu_half(t, 1)
                    else:
                        for ci in range(NT):
                            emit_hp(t, ci, 1)
                    # gates, natural orientation, per node-strip ai:
                    # gz[a, 0:3] = hp_strip.T @ U + xT_strip.T @ Xw
                    for ai in range(k * 4, (k + 1) * 4):
                        nc.tensor.matmul(
                            gps[:, ai * 3 : ai * 3 + 3],
                            t["hp"][:, ai * 128 : (ai + 1) * 128], U[:],
                            start=True, stop=False,
                        )
                        nc.tensor.matmul(
                            gps[:, ai * 3 : ai * 3 + 3],
                            t["xT"][:, ai * 128 : (ai + 1) * 128], Xw[:],
                            start=False, stop=True,
                        )
                    # sigmoid = 0.5*tanh(0.5 z) + 0.5, batched over 4 strips
                    sl = slice(k * 12, (k + 1) * 12)
                    nc.scalar.activation(
                        gn[:, sl], gps[:, sl], AF.Tanh, scale=0.5,
                    )
                    nc.vector.tensor_scalar(
                        gn[:, sl], gn[:, sl], 0.5, 0.5, OP.mult, OP.add,
                    )
                    # final elementwise stage for this half's node strips
                    for ai in range(k * 4, (k + 1) * 4):
                        nc.tensor.transpose(
                            hpn[:, ai, :],
                            t["hp"][:, ai * 128 : (ai + 1) * 128], I128[:],
                        )
                        nc.gpsimd.tensor_scalar(
                            v_all[:, ai, :], t["xn"][:, ai, :],
                            gn[:, ai * 3 + 1 : ai * 3 + 2], None, OP.mult,
                        )
                        nc.vector.scalar_tensor_tensor(
                            out=w_all[:, ai, :],
                            in0=hpn[:, ai, :],
                            scalar=gn[:, ai * 3 : ai * 3 + 1],
                            in1=v_all[:, ai, :],
                            op0=OP.mult, op1=OP.add,
                        )
                    nc.scalar.activation(
                        t_all[:, k * 4 : (k + 1) * 4, :],
                        w_all[:, k * 4 : (k + 1) * 4, :], AF.Tanh,
                    )
                    for ai in range(k * 4, (k + 1) * 4):
                        nc.gpsimd.tensor_scalar(
                            out_sb[:, ai, :], t_all[:, ai, :],
                            gn[:, ai * 3 + 2 : ai * 3 + 3], None, OP.mult,
                        )
                    nc.sync.dma_start(
                        out=out[g, :, k * 4 : (k + 1) * 4, :],
                        in_=out_sb[:, k * 4 : (k + 1) * 4, :],
                    )

            import contextlib
            loop_ctx = (tc.For_i(0, reps, 1) if reps > 1
                        else contextlib.nullcontext())
            with loop_ctx:
                # software pipeline across the per-core graphs: graph g+1's
                # setup is emitted inside graph g's strip loop (filling the
                # per-strip DVE/PE gaps) and both tails are emitted last, so
                # the in-order engine streams never idle on the previous
                # graph's tail dependencies
                for u in range(unroll):
                    ts = [None] * GPC
                    ts[0] = stage_a(0)
                    stage_b(ts[0], mid_hook=lambda: stage_a(1))
                    for g in range(1, GPC):
                        relu_half(ts[g - 1], 0)
                        ts[g] = ts[g - 1]["_next"]
                        hook = (lambda gg: lambda: stage_a(gg + 1))(g) \
                            if g + 1 < GPC else None
                        stage_b(ts[g], mid_hook=hook)
                    for g in range(GPC):
                        stage_c(ts[g], g, relu0_done=(g < GPC - 1))
    nc.finalize()
    return nc


_CACHE = {}


def _get_program():
    if "nc" not in _CACHE:
        _CACHE["nc"] = _build_program()
    return _CACHE["nc"]


def _make_consts(inputs):
    f16 = np.dtype("float16")
    Ww = np.asarray(inputs["Ww"], np.float64)
    Wb = np.asarray(inputs["Wb"], np.float64)
    A_ = np.asarray(inputs["A"], np.float64)
    As = A_ + A_.T
    C2 = As @ Ww
    b2 = As @ Wb
    ch = np.concatenate([
        Ww.T,                                  # C1T: lhsT for h = Ww @ xT
        C2.T,                                  # C2T: lhsT for hAs = C2 @ xT
        np.eye(D),
        50.0 * np.eye(D),
        np.stack([inputs["wi_u"], inputs["wf_u"], inputs["wo_u"]], axis=1),
        np.stack([inputs["wi_x"], inputs["wf_x"], inputs["wo_x"]], axis=1),
    ], axis=1).astype(f16)
    cf = np.stack(
        [Wb, b2, np.full(D, 0.5)], axis=1
    ).astype(np.float32)
    return np.ascontiguousarray(ch), np.ascontiguousarray(cf)


def _prep_inputs(inputs):
    """Host-side layout prep -> list of per-core input maps."""
    bf16 = mybir.dt.np(BF16)
    x = np.asarray(inputs["x"], np.float32)
    adj = np.asarray(inputs["adj"], np.float32)

    xT_all = np.ascontiguousarray(x.transpose(0, 2, 1)).astype(np.float16)
    # natural x in device layout [B, 128, NT, D]: [b, p, s, d] = x[b, s*128+p, d]
    xn_dev = np.ascontiguousarray(
        x.reshape(B, NT, 128, D).transpose(0, 2, 1, 3)).astype(np.float16)
    # (adj.T - 1) strips in device layout [B, 128, NT, N]:
    # [b, p, s, a] = adj[b, a, s*128+p] - 1   (values -1/0)
    adjm_dev = np.ascontiguousarray(
        (adj.transpose(0, 2, 1) - 1.0)
        .reshape(B, NT, 128, N).transpose(0, 2, 1, 3)
    ).astype(np.float16)
    deg = adj.sum(axis=1)  # deg[b, c] = number of nonzero adj[b, :, c]
    nz_dev = np.ascontiguousarray(
        (N - deg).reshape(B, NT, 128).transpose(0, 2, 1)).astype(np.float32)
    ch, cf = _make_consts(inputs)

    in_maps = []
    for c in range(NCORES):
        s = slice(c * GPC, (c + 1) * GPC)
        in_maps.append({
            "xT": xT_all[s],
            "xn": xn_dev[s],
            "adjm": adjm_dev[s],
            "nz": nz_dev[s],
            "consts_h": ch,
            "consts_f": cf,
        })
    return in_maps


def kernel(x, adj, Ww, Wb, A, wi_u, wi_x, wf_u, wf_x, wo_u, wo_x):
    inputs = {"x": x, "adj": adj, "Ww": Ww, "Wb": Wb, "A": A,
              "wi_u": wi_u, "wi_x": wi_x, "wf_u": wf_u, "wf_x": wf_x,
              "wo_u": wo_u, "wo_x": wo_x}
    in_maps = _prep_inputs(inputs)
    nc = _get_program()
    res = run_bass_kernel_spmd(nc, in_maps, list(range(NCORES)))
    out = np.empty((B, N, D), dtype=np.float32)
    for c in range(NCORES):
        # device layout [GPC, 128, NT, D] -> natural [GPC, N, D]
        dev = res.results[c]["out"]
        out[c * GPC : (c + 1) * GPC] = (
            dev.transpose(0, 2, 1, 3).reshape(GPC, N, D))
    return out


# revision 38
# speedup vs baseline: 18.5345x; 1.7376x over previous
"""GAT-with-LSTM-gates kernel for Trainium2, SPMD over 8 NeuronCores.

Problem: B=16 graphs, N=1024 nodes, D=128 features.
    h   = x @ Ww.T + Wb
    e   = (h @ A) @ h.T;  e_sym = e + e.T  (== h @ (A + A.T) @ h.T)
    s   = where(adj > 0, e_sym, 0)
    att = softmax(s, axis=1) * adj
    h'  = relu(att @ h)
    ic/fc/oc = sigmoid(h' @ w*_u + x @ w*_x)        (scalar per node)
    out = oc * tanh(ic * h' + fc * x)

Sharding: data-parallel over B; 2 graphs per core; params replicated.

Device-side formulation (per graph), transposed layouts so the softmax
axis is the free dimension.  All matmuls run at 1 PE cycle/row: fp16 for
the value path (score precision), bf16 where exp range is needed.
    hT[d, n]    = C1 @ xT + Wb           (C1 = Ww)
    hAsT[l, n]  = C2 @ xT + b2           (C2 = (A+A.T)@Ww, b2 = (A+A.T)@Wb,
                                          host-folded: no hT->hAsT dependency)
    e[c, a]     = hAsT_strip.T @ hT + 50*I @ (adjT-1)    (PSUM f32; the
                   second matmul injects an additive -50 mask so no
                   separate mask multiply is needed)
    q[c, a]     = exp(e)                 (ACT, bf16 out, range ~e^25; the
                   ACT accumulator emits Qs[c] = sum_a q[c, a] for free)
    Z[c]        = Qs[c] + (N - deg[c])   (nz host-precomputed)
    hs[c, d]    = h_nat[c, d] / Z[c]     (fused with the PSUM->SBUF move of
                   the hT-strip transposes; bf16: values can be ~1e-14)
    h'T[d, a]   = sum_c hs[c, d] q[c, a]    (PSUM accumulate over strips)
    gates (natural orientation, per node-strip, 3-wide matmuls):
        gz[a, j] = hp_strip.T @ U + xT_strip.T @ Xw
        gn       = 0.5 * tanh(0.5 * gz) + 0.5
    out[a, d]   = oc * tanh(ic * h'_nat + fc * x_nat)
"""

import numpy as np

import concourse.bacc as bacc
import concourse.bass as bass
import concourse.mybir as mybir
import concourse.tile as tile
from concourse.bass_utils import run_bass_kernel_spmd

F32 = mybir.dt.float32
F16 = mybir.dt.float16
F8 = mybir.dt.float8e4
BF16 = mybir.dt.bfloat16
AF = mybir.ActivationFunctionType
OP = mybir.AluOpType

B, N, D = 16, 1024, 128
NCORES = 8
GPC = B // NCORES  # graphs per core
NT = N // 128  # 8 column/row tiles of the [N, N] score matrix


def _build_program(reps=1, unroll=1):
    """reps>1 wraps the whole per-call body in a hardware loop — used only
    for benchmarking (amortizes the host->device dispatch overhead).
    unroll>1 repeats the body inline (used by the timeline simulator to
    estimate the steady-state period, since it cannot follow For_i)."""
    nc = bacc.Bacc(None, enable_partition_id=False)

    xT = nc.dram_tensor("xT", [GPC, D, N], F16, kind="ExternalInput")
    xn = nc.dram_tensor("xn", [GPC, 128, NT, D], F16, kind="ExternalInput")
    nzd = nc.dram_tensor("nz", [GPC, 128, NT], F32, kind="ExternalInput")
    # adjm = adj.T - 1 (values -1/0): the e-score mask is injected on the PE
    # as e += 50*I @ adjm, i.e. -50 on masked entries
    adjm = nc.dram_tensor("adjm", [GPC, 128, NT, N], F8, kind="ExternalInput")
    # replicated params: fp16 [C1T | C2T | I128 | 50*I | U | Xw], f32 [Wb | b2]
    ch = nc.dram_tensor("consts_h", [D, 518], F16, kind="ExternalInput")
    cf = nc.dram_tensor("consts_f", [D, 3], F32, kind="ExternalInput")
    cb = nc.dram_tensor("consts_b", [D, 128], F8, kind="ExternalInput")
    out = nc.dram_tensor("out", [GPC, 128, NT, D], F16, kind="ExternalOutput")

    with tile.TileContext(nc) as tc:
        with (
            tc.tile_pool(name="const", bufs=1) as constp,
            tc.tile_pool(name="big", bufs=2) as big,
            tc.tile_pool(name="adjp", bufs=2) as adjp,
            tc.tile_pool(name="small", bufs=2) as small,
            tc.tile_pool(name="ps_a", bufs=4, space="PSUM") as ps_a,
            tc.tile_pool(name="ps_b", bufs=2, space="PSUM") as ps_b,
        ):
            consts = constp.tile([D, 518], F16, name="ch_sb")
            nc.sync.dma_start(out=consts[:], in_=ch[:])
            constf = constp.tile([D, 3], F32, name="cf_sb")
            nc.sync.dma_start(out=constf[:], in_=cf[:])
            I48 = constp.tile([D, 128], F8, name="cb_sb")
            nc.sync.dma_start(out=I48[:], in_=cb[:])
            C1T = consts[:, 0:128]
            C2T = consts[:, 128:256]
            I128 = consts[:, 256:384]
            U = consts[:, 512:515]
            Xw = consts[:, 515:518]
            Wb = constf[:, 0:1]
            b2 = constf[:, 1:2]

            def half_mm(dst2, lhsT, src, name):
                """matmul into two 1-bank PSUM tiles (tag psa), one per
                512-col half; returns the pair."""
                for k in range(2):
                    nc.tensor.matmul(
                        dst2[k][:], lhsT,
                        src[:, k * 512 : (k + 1) * 512],
                        start=True, stop=True,
                    )

            def stage_a(g):
                """DMAs, hT/hAsT, h-natural transposes."""
                t = {}
                t["xT"] = big.tile([D, N], F16, name="xT_sb", tag="xT")
                nc.sync.dma_start(out=t["xT"][:], in_=xT[g])
                t["adj"] = adjp.tile([128, NT, N], F8, name="adj_sb", tag="adj")
                nc.sync.dma_start(out=t["adj"][:], in_=adjm[g])
                t["nz"] = small.tile([128, NT], F32, name="nz", tag="nz")
                nc.sync.dma_start(out=t["nz"][:], in_=nzd[g])
                t["xn"] = big.tile([128, NT, D], F16, name="xn_sb", tag="xn")
                nc.sync.dma_start(out=t["xn"][:], in_=xn[g])

                hT_ps = [ps_a.tile([D, 512], F32, name=f"hT_ps{k}", tag="psa")
                         for k in range(2)]
                half_mm(hT_ps, C1T[:], t["xT"], "hT")
                t["hT"] = big.tile([D, N], F16, name="hT", tag="hT")
                for k in range(2):
                    nc.vector.tensor_scalar(
                        t["hT"][:, k * 512 : (k + 1) * 512], hT_ps[k][:],
                        Wb[:], None, OP.add,
                    )
                hA_ps = [ps_a.tile([D, 512], F32, name=f"hA_ps{k}", tag="psa")
                         for k in range(2)]
                half_mm(hA_ps, C2T[:], t["xT"], "hA")
                t["hAsT"] = big.tile([D, N], F16, name="hAsT", tag="hAsT")
                for k in range(2):
                    nc.vector.tensor_scalar(
                        t["hAsT"][:, k * 512 : (k + 1) * 512], hA_ps[k][:],
                        b2[:], None, OP.add,
                    )
                # h natural (lhsT of the h' contraction): the PSUM->SBUF
                # move is fused with the 1/Z scale in stage_b
                t["tr"] = ps_b.tile([128, NT, D], F16, name="tr_ps", tag="trh")
                for ci in range(NT):
                    nc.tensor.transpose(
                        t["tr"][:, ci, :],
                        t["hT"][:, ci * 128 : (ci + 1) * 128], I128[:],
                    )
                return t

            def emit_hp(t, ci):
                for k in range(2):
                    nc.tensor.matmul(
                        t["hp_ps"][:, k, :],
                        t["hs"][:, ci, :],
                        t["q"][:, ci, k * 512 : (k + 1) * 512],
                        start=(ci == 0), stop=(ci == NT - 1),
                    )

            def stage_b(t, mid_hook=None):
                """Attention strips; h' matmuls one strip behind.
                mid_hook() is emitted after strip 2 so the next graph's
                setup work fills this graph's per-strip engine gaps."""
                t["Qs"] = small.tile([128, NT], F32, name="Qs", tag="Qs")
                t["Z"] = small.tile([128, NT], F32, name="Z", tag="Z")
                t["R"] = small.tile([128, NT], F32, name="R", tag="R")
                t["hs"] = big.tile([128, NT, D], BF16, name="hs", tag="hs")
                t["qs2"] = big.tile([128, N], BF16, name="qs2", tag="qs2")
                t["q"] = big.tile([128, NT, N], BF16, name="q_sb", tag="q")
                t["hp_ps"] = ps_b.tile([D, 2, 512], F32, name="hp_ps",
                                       tag="pshp", bufs=1)
                t["hp"] = big.tile([D, N], F16, name="hp", tag="hp")
                for ci in range(NT):
                    e_ps = [ps_a.tile([128, 512], F32, name=f"e_ps{k}", tag="psa")
                            for k in range(2)]
                    for k in range(2):
                        nc.tensor.matmul(
                            e_ps[k][:],
                            t["hAsT"][:, ci * 128 : (ci + 1) * 128],
                            t["hT"][:, k * 512 : (k + 1) * 512],
                            start=True, stop=False,
                        )
                        # additive mask: e += 48*I @ (adjT-1)
                        nc.tensor.matmul(
                            e_ps[k][:],
                            I48[:],
                            t["adj"][:, ci, k * 512 : (k + 1) * 512],
                            start=False, stop=True, skip_group_check=True,
                        )
                    if ci > 0:
                        emit_hp(t, ci - 1)
                    # q = exp(e - 48*(1-adj)), one ACT op per PSUM half
                    for k in range(2):
                        nc.scalar.activation(
                            t["q"][:, ci, k * 512 : (k + 1) * 512],
                            e_ps[k][:], AF.Exp,
                        )
                    # row-sum of q on DVE (4x-mode copy w/ accumulate);
                    # Z / 1/Z / hs stay on DVE: same-engine chains are cheap
                    nc.vector.tensor_scalar(
                        t["qs2"][:], t["q"][:, ci, :], 1.0, 0.0,
                        OP.mult, OP.add,
                        accum_out=t["Qs"][:, ci : ci + 1],
                    )
                    nc.vector.tensor_scalar(
                        t["Z"][:, ci : ci + 1], t["Qs"][:, ci : ci + 1],
                        t["nz"][:, ci : ci + 1], None, OP.add,
                    )
                    nc.vector.reciprocal(
                        t["R"][:, ci : ci + 1], t["Z"][:, ci : ci + 1])
                    nc.vector.tensor_scalar(
                        t["hs"][:, ci, :], t["tr"][:, ci, :],
                        t["R"][:, ci : ci + 1], None, OP.mult,
                    )
                    if ci == 2 and mid_hook is not None:
                        t["_next"] = mid_hook()
                emit_hp(t, NT - 1)

            def relu(t):
                nc.vector.tensor_scalar(
                    t["hp"].rearrange("p (a b) -> p a b", a=2), t["hp_ps"][:],
                    0.0, None, OP.max,
                )

            def stage_c(t, g):
                """Gates + final elementwise stage + output, organized as
                single-engine instruction chains (cross-engine dependency
                hops are expensive on this runtime)."""
                gps = ps_b.tile([128, 32], F32, name="gps", tag="pshp", bufs=1)
                gn = small.tile([128, 3 * NT], F32, name="gn", tag="gn")
                hpn = ps_b.tile([128, NT, D], F16, name="hpn", tag="trh")
                v_all = big.tile([128, NT, D], F16, name="v_all", tag="v")
                w_all = big.tile([128, NT, D], F16, name="w_all", tag="w")
                t_all = big.tile([128, NT, D], F16, name="t_all", tag="t")
                out_sb = big.tile([128, NT, D], F16, name="out_sb", tag="o")
                # gates, natural orientation: gz[a, j] = hp.T @ U + xT.T @ Xw
                for ai in range(NT):
                    nc.tensor.matmul(
                        gps[:, ai * 3 : ai * 3 + 3],
                        t["hp"][:, ai * 128 : (ai + 1) * 128], U[:],
                        start=True, stop=False,
                    )
                    nc.tensor.matmul(
                        gps[:, ai * 3 : ai * 3 + 3],
                        t["xT"][:, ai * 128 : (ai + 1) * 128], Xw[:],
                        start=False, stop=True,
                    )
                for ai in range(NT):
                    nc.tensor.transpose(
                        hpn[:, ai, :],
                        t["hp"][:, ai * 128 : (ai + 1) * 128], I128[:],
                    )
                # sigmoid = 0.5*tanh(0.5 z) + 0.5 over all 8 strips at once
                nc.scalar.activation(
                    gn[:], gps[:, 0:24], AF.Tanh, scale=0.5,
                )
                nc.vector.tensor_scalar(
                    gn[:], gn[:], 0.5, 0.5, OP.mult, OP.add,
                )
                for ai in range(NT):
                    nc.vector.tensor_scalar(
                        v_all[:, ai, :], t["xn"][:, ai, :],
                        gn[:, ai * 3 + 1 : ai * 3 + 2], None, OP.mult,
                    )
                    nc.vector.scalar_tensor_tensor(
                        out=w_all[:, ai, :],
                        in0=hpn[:, ai, :],
                        scalar=gn[:, ai * 3 : ai * 3 + 1],
                        in1=v_all[:, ai, :],
                        op0=OP.mult, op1=OP.add,
                    )
                nc.scalar.activation(t_all[:], w_all[:], AF.Tanh)
                for ai in range(NT):
                    nc.vector.tensor_scalar(
                        out_sb[:, ai, :], t_all[:, ai, :],
                        gn[:, ai * 3 + 2 : ai * 3 + 3], None, OP.mult,
                    )
                nc.sync.dma_start(out=out[g], in_=out_sb[:])

            import contextlib
            loop_ctx = (tc.For_i(0, reps, 1) if reps > 1
                        else contextlib.nullcontext())
            with loop_ctx:
                # software pipeline across the per-core graphs: graph g+1's
                # setup is emitted inside graph g's strip loop, all relus
                # before any tail, and tails last — the in-order engine
                # streams then never head-of-line block on tail work
                for u in range(unroll):
                    ts = [None] * GPC
                    ts[0] = stage_a(0)
                    stage_b(ts[0], mid_hook=lambda: stage_a(1))
                    for g in range(1, GPC):
                        relu(ts[g - 1])
                        ts[g] = ts[g - 1]["_next"]
                        hook = (lambda gg: lambda: stage_a(gg + 1))(g) \
                            if g + 1 < GPC else None
                        stage_b(ts[g], mid_hook=hook)
                    relu(ts[GPC - 1])
                    for g in range(GPC):
                        stage_c(ts[g], g)
    nc.finalize()
    return nc


_CACHE = {}


def _get_program():
    if "nc" not in _CACHE:
        _CACHE["nc"] = _build_program()
    return _CACHE["nc"]


def _make_consts(inputs):
    f16 = np.dtype("float16")
    Ww = np.asarray(inputs["Ww"], np.float64)
    Wb = np.asarray(inputs["Wb"], np.float64)
    A_ = np.asarray(inputs["A"], np.float64)
    As = A_ + A_.T
    C2 = As @ Ww
    b2 = As @ Wb
    ch = np.concatenate([
        Ww.T,                                  # C1T: lhsT for h = Ww @ xT
        C2.T,                                  # C2T: lhsT for hAs = C2 @ xT
        np.eye(D),
        50.0 * np.eye(D),
        np.stack([inputs["wi_u"], inputs["wf_u"], inputs["wo_u"]], axis=1),
        np.stack([inputs["wi_x"], inputs["wf_x"], inputs["wo_x"]], axis=1),
    ], axis=1).astype(f16)
    cf = np.stack(
        [Wb, b2, np.full(D, 0.5)], axis=1
    ).astype(np.float32)
    cb = (48.0 * np.eye(D)).astype(mybir.dt.np(F8))
    return (np.ascontiguousarray(ch), np.ascontiguousarray(cf),
            np.ascontiguousarray(cb))


def _prep_inputs(inputs):
    """Host-side layout prep -> list of per-core input maps."""
    bf16 = mybir.dt.np(BF16)
    x = np.asarray(inputs["x"], np.float32)
    adj = np.asarray(inputs["adj"], np.float32)

    xT_all = np.ascontiguousarray(x.transpose(0, 2, 1)).astype(np.float16)
    # natural x in device layout [B, 128, NT, D]: [b, p, s, d] = x[b, s*128+p, d]
    xn_dev = np.ascontiguousarray(
        x.reshape(B, NT, 128, D).transpose(0, 2, 1, 3)).astype(np.float16)
    # (adj.T - 1) strips in device layout [B, 128, NT, N]:
    # [b, p, s, a] = adj[b, a, s*128+p] - 1   (values -1/0)
    adjm_dev = np.ascontiguousarray(
        (adj.transpose(0, 2, 1) - 1.0)
        .reshape(B, NT, 128, N).transpose(0, 2, 1, 3)
    ).astype(mybir.dt.np(F8))
    deg = adj.sum(axis=1)  # deg[b, c] = number of nonzero adj[b, :, c]
    nz_dev = np.ascontiguousarray(
        (N - deg).reshape(B, NT, 128).transpose(0, 2, 1)).astype(np.float32)
    ch, cf, cb = _make_consts(inputs)

    in_maps = []
    for c in range(NCORES):
        s = slice(c * GPC, (c + 1) * GPC)
        in_maps.append({
            "xT": xT_all[s],
            "xn": xn_dev[s],
            "adjm": adjm_dev[s],
            "nz": nz_dev[s],
            "consts_h": ch,
            "consts_f": cf,
            "consts_b": cb,
        })
    return in_maps


def kernel(x, adj, Ww, Wb, A, wi_u, wi_x, wf_u, wf_x, wo_u, wo_x):
    inputs = {"x": x, "adj": adj, "Ww": Ww, "Wb": Wb, "A": A,
              "wi_u": wi_u, "wi_x": wi_x, "wf_u": wf_u, "wf_x": wf_x,
              "wo_u": wo_u, "wo_x": wo_x}
    in_maps = _prep_inputs(inputs)
    nc = _get_program()
    res = run_bass_kernel_spmd(nc, in_maps, list(range(NCORES)))
    out = np.empty((B, N, D), dtype=np.float32)
    for c in range(NCORES):
        # device layout [GPC, 128, NT, D] -> natural [GPC, N, D]
        dev = np.asarray(res.results[c]["out"], dtype=np.float32)
        out[c * GPC : (c + 1) * GPC] = (
            dev.transpose(0, 2, 1, 3).reshape(GPC, N, D))
    return out


# revision 41
# speedup vs baseline: 19.8470x; 1.0708x over previous
"""GAT-with-LSTM-gates kernel for Trainium2, SPMD over 8 NeuronCores.

Problem: B=16 graphs, N=1024 nodes, D=128 features.
    h   = x @ Ww.T + Wb
    e   = (h @ A) @ h.T;  e_sym = e + e.T  (== h @ (A + A.T) @ h.T)
    s   = where(adj > 0, e_sym, 0)
    att = softmax(s, axis=1) * adj
    h'  = relu(att @ h)
    ic/fc/oc = sigmoid(h' @ w*_u + x @ w*_x)        (scalar per node)
    out = oc * tanh(ic * h' + fc * x)

Sharding: data-parallel over B; 2 graphs per core; params replicated.

Device-side formulation (per graph), transposed layouts so the softmax
axis is the free dimension.  All matmuls run at 1 PE cycle/row: fp16 for
the value path (score precision), bf16 where exp range is needed, fp8
for the adjacency mask (values -1/0 and the 48*I mask scale are exact).
    hT[d, n]    = C1 @ xT + Wb           (C1 = Ww)
    hAsT[l, n]  = C2 @ xT + b2           (C2 = (A+A.T)@Ww, b2 = (A+A.T)@Wb,
                                          host-folded: no hT->hAsT dependency)
    e[c, a]     = hAsT_strip.T @ hT + 48*I @ (adjT-1)    (PSUM f32; the
                   second matmul injects an additive -48 mask so no
                   separate mask multiply is needed)
    q[c, a]     = exp(e)                 (ACT, bf16 out, range ~e^25)
    Z[c]        = sum_a q[c, a] + (N - deg[c])   (row-sums via a 4x-mode
                   DVE copy-with-accumulate; nz host-precomputed)
    hs[c, d]    = h_nat[c, d] / Z[c]     (fused with the PSUM->SBUF move of
                   the hT-strip transposes; bf16: values can be ~1e-14)
    h'T[d, a]   = sum_c hs[c, d] q[c, a]    (PSUM accumulate over strips)
    gates (natural orientation, per node-strip, 3-wide matmuls):
        gz[a, j] = hp_strip.T @ U + xT_strip.T @ Xw
        gn       = 0.5 * tanh(0.5 * gz) + 0.5
    out[a, d]   = oc * tanh(ic * h'_nat + fc * x_nat)

Scheduling notes (dominant costs on this runtime, found empirically):
cross-engine dependency hops and instruction chains cost far more than
engine throughput, so serial chains are kept on a single engine (DVE),
the two graphs are software-pipelined at emission level, and all tiles
are multi-buffered so no pool recycling sits on the critical path.
"""

import numpy as np

import concourse.bacc as bacc
import concourse.mybir as mybir
import concourse.tile as tile
from concourse.bass_utils import run_bass_kernel_spmd

F32 = mybir.dt.float32
F16 = mybir.dt.float16
F8 = mybir.dt.float8e4
BF16 = mybir.dt.bfloat16
AF = mybir.ActivationFunctionType
OP = mybir.AluOpType

B, N, D = 16, 1024, 128
NCORES = 8
GPC = B // NCORES  # graphs per core
NT = N // 128  # 8 column/row tiles of the [N, N] score matrix


def _build_program(reps=1, unroll=1):
    """reps>1 wraps the whole per-call body in a hardware loop — used only
    for benchmarking (amortizes the host->device dispatch overhead).
    unroll>1 repeats the body inline (used by the timeline simulator to
    estimate the steady-state period, since it cannot follow For_i)."""
    nc = bacc.Bacc(None, enable_partition_id=False)

    xT = nc.dram_tensor("xT", [GPC, D, N], F16, kind="ExternalInput")
    xn = nc.dram_tensor("xn", [GPC, 128, NT, D], F16, kind="ExternalInput")
    nzd = nc.dram_tensor("nz", [GPC, 128, NT], F32, kind="ExternalInput")
    # adjm = adj.T - 1 (values -1/0): the e-score mask is injected on the PE
    # as e += 48*I @ adjm, i.e. -48 on masked entries
    adjm = nc.dram_tensor("adjm", [GPC, 128, NT, N], F8, kind="ExternalInput")
    # replicated params: fp16 [C1T | C2T | I128 | (pad) | U | Xw], f32 [Wb | b2]
    ch = nc.dram_tensor("consts_h", [D, 518], F16, kind="ExternalInput")
    cf = nc.dram_tensor("consts_f", [D, 3], F32, kind="ExternalInput")
    cb = nc.dram_tensor("consts_b", [D, 128], F8, kind="ExternalInput")
    out = nc.dram_tensor("out", [GPC, 128, NT, D], F16, kind="ExternalOutput")

    with tile.TileContext(nc) as tc:
        with (
            tc.tile_pool(name="const", bufs=1) as constp,
            tc.tile_pool(name="big", bufs=2) as big,
            tc.tile_pool(name="adjp", bufs=2) as adjp,
            tc.tile_pool(name="small", bufs=2) as small,
            tc.tile_pool(name="ps_a", bufs=4, space="PSUM") as ps_a,
            tc.tile_pool(name="ps_b", bufs=2, space="PSUM") as ps_b,
        ):
            consts = constp.tile([D, 518], F16, name="ch_sb")
            nc.sync.dma_start(out=consts[:], in_=ch[:])
            constf = constp.tile([D, 3], F32, name="cf_sb")
            nc.sync.dma_start(out=constf[:], in_=cf[:])
            I48 = constp.tile([D, 128], F8, name="cb_sb")
            nc.sync.dma_start(out=I48[:], in_=cb[:])
            C1T = consts[:, 0:128]
            C2T = consts[:, 128:256]
            I128 = consts[:, 256:384]
            U = consts[:, 512:515]
            Xw = consts[:, 515:518]
            Wb = constf[:, 0:1]
            b2 = constf[:, 1:2]

            def half_mm(dst2, lhsT, src, name):
                """matmul into two 1-bank PSUM tiles (tag psa), one per
                512-col half; returns the pair."""
                for k in range(2):
                    nc.tensor.matmul(
                        dst2[k][:], lhsT,
                        src[:, k * 512 : (k + 1) * 512],
                        start=True, stop=True,
                    )

            def stage_a(g):
                """DMAs, hT/hAsT, h-natural transposes."""
                t = {}
                t["xT"] = big.tile([D, N], F16, name="xT_sb", tag="xT")
                nc.sync.dma_start(out=t["xT"][:], in_=xT[g])
                t["adj"] = adjp.tile([128, NT, N], F8, name="adj_sb", tag="adj")
                nc.sync.dma_start(out=t["adj"][:], in_=adjm[g])
                t["nz"] = small.tile([128, NT], F32, name="nz", tag="nz")
                nc.sync.dma_start(out=t["nz"][:], in_=nzd[g])
                t["xn"] = big.tile([128, NT, D], F16, name="xn_sb", tag="xn")
                nc.sync.dma_start(out=t["xn"][:], in_=xn[g])

                hT_ps = [ps_a.tile([D, 512], F32, name=f"hT_ps{k}", tag="psa")
                         for k in range(2)]
                half_mm(hT_ps, C1T[:], t["xT"], "hT")
                t["hT"] = big.tile([D, N], F16, name="hT", tag="hT")
                for k in range(2):
                    nc.vector.tensor_scalar(
                        t["hT"][:, k * 512 : (k + 1) * 512], hT_ps[k][:],
                        Wb[:], None, OP.add,
                    )
                hA_ps = [ps_a.tile([D, 512], F32, name=f"hA_ps{k}", tag="psa")
                         for k in range(2)]
                half_mm(hA_ps, C2T[:], t["xT"], "hA")
                t["hAsT"] = big.tile([D, N], F16, name="hAsT", tag="hAsT")
                for k in range(2):
                    nc.vector.tensor_scalar(
                        t["hAsT"][:, k * 512 : (k + 1) * 512], hA_ps[k][:],
                        b2[:], None, OP.add,
                    )
                # h natural (lhsT of the h' contraction): the PSUM->SBUF
                # move is fused with the 1/Z scale in stage_b
                t["tr"] = ps_b.tile([128, NT, D], F16, name="tr_ps", tag="trh")
                for ci in range(NT):
                    nc.tensor.transpose(
                        t["tr"][:, ci, :],
                        t["hT"][:, ci * 128 : (ci + 1) * 128], I128[:],
                    )
                return t

            def emit_hp(t, ci):
                for k in range(2):
                    nc.tensor.matmul(
                        t["hp_ps"][:, k, :],
                        t["hs"][:, ci, :],
                        t["q"][:, ci, k * 512 : (k + 1) * 512],
                        start=(ci == 0), stop=(ci == NT - 1),
                    )

            def stage_b(t, mid_hook=None):
                """Attention strips; h' matmuls one strip behind.
                mid_hook() is emitted after strip 2 so the next graph's
                setup work fills this graph's per-strip engine gaps."""
                t["Qs"] = small.tile([128, NT], F32, name="Qs", tag="Qs")
                t["Z"] = small.tile([128, NT], F32, name="Z", tag="Z")
                t["R"] = small.tile([128, NT], F32, name="R", tag="R")
                t["hs"] = big.tile([128, NT, D], BF16, name="hs", tag="hs")
                t["qs2"] = big.tile([128, N], BF16, name="qs2", tag="qs2")
                t["q"] = big.tile([128, NT, N], BF16, name="q_sb", tag="q")
                t["hp_ps"] = ps_b.tile([D, 2, 512], F32, name="hp_ps",
                                       tag="pshp", bufs=1)
                t["hp"] = big.tile([D, N], F16, name="hp", tag="hp")
                for ci in range(NT):
                    e_ps = [ps_a.tile([128, 512], F32, name=f"e_ps{k}", tag="psa")
                            for k in range(2)]
                    for k in range(2):
                        nc.tensor.matmul(
                            e_ps[k][:],
                            t["hAsT"][:, ci * 128 : (ci + 1) * 128],
                            t["hT"][:, k * 512 : (k + 1) * 512],
                            start=True, stop=False,
                        )
                        # additive mask: e += 48*I @ (adjT-1)
                        nc.tensor.matmul(
                            e_ps[k][:],
                            I48[:],
                            t["adj"][:, ci, k * 512 : (k + 1) * 512],
                            start=False, stop=True, skip_group_check=True,
                        )
                    if ci > 0:
                        emit_hp(t, ci - 1)
                    # q = exp(e - 48*(1-adj)), one ACT op per PSUM half
                    for k in range(2):
                        nc.scalar.activation(
                            t["q"][:, ci, k * 512 : (k + 1) * 512],
                            e_ps[k][:], AF.Exp,
                        )
                    # row-sum of q on DVE (4x-mode copy w/ accumulate);
                    # Z / 1/Z / hs stay on DVE: same-engine chains are cheap
                    nc.vector.tensor_scalar(
                        t["qs2"][:], t["q"][:, ci, :], 1.0, 0.0,
                        OP.mult, OP.add,
                        accum_out=t["Qs"][:, ci : ci + 1],
                    )
                    nc.vector.tensor_scalar(
                        t["Z"][:, ci : ci + 1], t["Qs"][:, ci : ci + 1],
                        t["nz"][:, ci : ci + 1], None, OP.add,
                    )
                    nc.vector.reciprocal(
                        t["R"][:, ci : ci + 1], t["Z"][:, ci : ci + 1])
                    nc.vector.tensor_scalar(
                        t["hs"][:, ci, :], t["tr"][:, ci, :],
                        t["R"][:, ci : ci + 1], None, OP.mult,
                    )
                    if ci == 2 and mid_hook is not None:
                        t["_next"] = mid_hook()
                emit_hp(t, NT - 1)

            def relu(t):
                nc.vector.tensor_scalar(
                    t["hp"].rearrange("p (a b) -> p a b", a=2), t["hp_ps"][:],
                    0.0, None, OP.max,
                )

            def stage_c(t, g):
                """Gates + final elementwise stage + output, organized as
                single-engine instruction chains (cross-engine dependency
                hops are expensive on this runtime)."""
                gps = ps_b.tile([128, 32], F32, name="gps", tag="pshp", bufs=1)
                gn = small.tile([128, 3 * NT], F32, name="gn", tag="gn")
                hpn = ps_b.tile([128, NT, D], F16, name="hpn", tag="trh")
                v_all = big.tile([128, NT, D], F16, name="v_all", tag="v")
                w_all = big.tile([128, NT, D], F16, name="w_all", tag="w")
                t_all = big.tile([128, NT, D], F16, name="t_all", tag="t")
                out_sb = big.tile([128, NT, D], F16, name="out_sb", tag="o")
                # gates, natural orientation: gz[a, j] = hp.T @ U + xT.T @ Xw
                for ai in range(NT):
                    nc.tensor.matmul(
                        gps[:, ai * 3 : ai * 3 + 3],
                        t["hp"][:, ai * 128 : (ai + 1) * 128], U[:],
                        start=True, stop=False,
                    )
                    nc.tensor.matmul(
                        gps[:, ai * 3 : ai * 3 + 3],
                        t["xT"][:, ai * 128 : (ai + 1) * 128], Xw[:],
                        start=False, stop=True,
                    )
                for ai in range(NT):
                    nc.tensor.transpose(
                        hpn[:, ai, :],
                        t["hp"][:, ai * 128 : (ai + 1) * 128], I128[:],
                    )
                # sigmoid = 0.5*tanh(0.5 z) + 0.5 over all 8 strips at once
                nc.scalar.activation(
                    gn[:], gps[:, 0:24], AF.Tanh, scale=0.5,
                )
                nc.vector.tensor_scalar(
                    gn[:], gn[:], 0.5, 0.5, OP.mult, OP.add,
                )
                for ai in range(NT):
                    nc.vector.tensor_scalar(
                        v_all[:, ai, :], t["xn"][:, ai, :],
                        gn[:, ai * 3 + 1 : ai * 3 + 2], None, OP.mult,
                    )
                    nc.vector.scalar_tensor_tensor(
                        out=w_all[:, ai, :],
                        in0=hpn[:, ai, :],
                        scalar=gn[:, ai * 3 : ai * 3 + 1],
                        in1=v_all[:, ai, :],
                        op0=OP.mult, op1=OP.add,
                    )
                nc.scalar.activation(t_all[:], w_all[:], AF.Tanh)
                for ai in range(NT):
                    nc.vector.tensor_scalar(
                        out_sb[:, ai, :], t_all[:, ai, :],
                        gn[:, ai * 3 + 2 : ai * 3 + 3], None, OP.mult,
                    )
                nc.sync.dma_start(out=out[g], in_=out_sb[:])

            import contextlib
            loop_ctx = (tc.For_i(0, reps, 1) if reps > 1
                        else contextlib.nullcontext())
            with loop_ctx:
                # software pipeline across the per-core graphs: graph g+1's
                # setup is emitted inside graph g's strip loop, all relus
                # before any tail, and tails last — the in-order engine
                # streams then never head-of-line block on tail work
                for u in range(unroll):
                    ts = [None] * GPC
                    ts[0] = stage_a(0)
                    stage_b(ts[0], mid_hook=lambda: stage_a(1))
                    for g in range(1, GPC):
                        relu(ts[g - 1])
                        ts[g] = ts[g - 1]["_next"]
                        hook = (lambda gg: lambda: stage_a(gg + 1))(g) \
                            if g + 1 < GPC else None
                        stage_b(ts[g], mid_hook=hook)
                    relu(ts[GPC - 1])
                    for g in range(GPC):
                        stage_c(ts[g], g)
    nc.finalize()
    return nc


_CACHE = {}


def _get_program():
    if "nc" not in _CACHE:
        _CACHE["nc"] = _build_program()
    return _CACHE["nc"]


def _make_consts(inputs):
    f16 = np.dtype("float16")
    Ww = np.asarray(inputs["Ww"], np.float64)
    Wb = np.asarray(inputs["Wb"], np.float64)
    A_ = np.asarray(inputs["A"], np.float64)
    As = A_ + A_.T
    C2 = As @ Ww
    b2 = As @ Wb
    ch = np.concatenate([
        Ww.T,                                  # C1T: lhsT for h = Ww @ xT
        C2.T,                                  # C2T: lhsT for hAs = C2 @ xT
        np.eye(D),
        50.0 * np.eye(D),
        np.stack([inputs["wi_u"], inputs["wf_u"], inputs["wo_u"]], axis=1),
        np.stack([inputs["wi_x"], inputs["wf_x"], inputs["wo_x"]], axis=1),
    ], axis=1).astype(f16)
    cf = np.stack(
        [Wb, b2, np.full(D, 0.5)], axis=1
    ).astype(np.float32)
    cb = (48.0 * np.eye(D)).astype(mybir.dt.np(F8))
    return (np.ascontiguousarray(ch), np.ascontiguousarray(cf),
            np.ascontiguousarray(cb))


def _prep_inputs(inputs):
    """Host-side layout prep -> list of per-core input maps."""
    bf16 = mybir.dt.np(BF16)
    x = np.asarray(inputs["x"], np.float32)
    adj = np.asarray(inputs["adj"], np.float32)

    xT_all = np.ascontiguousarray(x.transpose(0, 2, 1)).astype(np.float16)
    # natural x in device layout [B, 128, NT, D]: [b, p, s, d] = x[b, s*128+p, d]
    xn_dev = np.ascontiguousarray(
        x.reshape(B, NT, 128, D).transpose(0, 2, 1, 3)).astype(np.float16)
    # (adj.T - 1) strips in device layout [B, 128, NT, N]:
    # [b, p, s, a] = adj[b, a, s*128+p] - 1   (values -1/0)
    adjm_dev = np.ascontiguousarray(
        (adj.transpose(0, 2, 1) - 1.0)
        .reshape(B, NT, 128, N).transpose(0, 2, 1, 3)
    ).astype(mybir.dt.np(F8))
    deg = adj.sum(axis=1)  # deg[b, c] = number of nonzero adj[b, :, c]
    nz_dev = np.ascontiguousarray(
        (N - deg).reshape(B, NT, 128).transpose(0, 2, 1)).astype(np.float32)
    ch, cf, cb = _make_consts(inputs)

    in_maps = []
    for c in range(NCORES):
        s = slice(c * GPC, (c + 1) * GPC)
        in_maps.append({
            "xT": xT_all[s],
            "xn": xn_dev[s],
            "adjm": adjm_dev[s],
            "nz": nz_dev[s],
            "consts_h": ch,
            "consts_f": cf,
            "consts_b": cb,
        })
    return in_maps


def kernel(x, adj, Ww, Wb, A, wi_u, wi_x, wf_u, wf_x, wo_u, wo_x):
    inputs = {"x": x, "adj": adj, "Ww": Ww, "Wb": Wb, "A": A,
              "wi_u": wi_u, "wi_x": wi_x, "wf_u": wf_u, "wf_x": wf_x,
              "wo_u": wo_u, "wo_x": wo_x}
    in_maps = _prep_inputs(inputs)
    nc = _get_program()
    res = run_bass_kernel_spmd(nc, in_maps, list(range(NCORES)))
    out = np.empty((B, N, D), dtype=np.float32)
    for c in range(NCORES):
        # device layout [GPC, 128, NT, D] -> natural [GPC, N, D]
        dev = np.asarray(res.results[c]["out"], dtype=np.float32)
        out[c * GPC : (c + 1) * GPC] = (
            dev.transpose(0, 2, 1, 3).reshape(GPC, N, D))
    return out
